# revision 24
# baseline (speedup 1.0000x reference)
"""Trainium2 Bass kernel for nn_ConvchannelAttentionBlock.

reference (per batch b):
    S      = x @ x.T                      (C x C, symmetric; contraction over L)
    probs  = softmax(rowmax(S) - S)       (shift-invariant: == softmax(-S))
    read   = probs @ x                    (C x L)
    out    = eta * read + x

Sharding: data-parallel over B. Each of the 8 cores gets 4 batches and
runs an identical NEFF (SPMD) on its shard; outputs are concatenated.

Key structure (v2):
  * x is staged to DRAM in bf16 (working precision); output is written
    bf16 and upcast on the host. Residual error ~0.3% rel, well under
    the 2e-2 gate.
  * xT (the [L, C]-layout copy needed for the S = xT.T @ xT contraction
    over L) is loaded straight from DRAM with the DMA xbar transpose --
    zero PE-transpose / zero cast work on-chip.
  * mm1 computes only upper-triangular blocks of S (symmetric); S stays
    in f32 PSUM.  Lower blocks are mirrored with 6 small PE transposes
    per batch (via a bf16 staging copy of the off-diagonal blocks).
  * softmax: rowmin over the PSUM regions on DVE; E = exp(rowmin - S)
    reads PSUM f32 directly on ACT with fused row-sum accumulators
    (Z comes for free); s = eta/Z on DVE.
  * E^T for mm2's stationary operand via 16 PE transposes per batch.
  * mm2 epilogue: out = s * R + x with s = eta/Z, the per-tile mul/add
    split across ACT/DVE/GPSIMD; result staged bf16, DMA'd in 1MB rows.
"""

import sys

if "/opt/trn_rl_repo" not in sys.path:
    sys.path.insert(0, "/opt/trn_rl_repo")

import numpy as np
import ml_dtypes

import concourse.bacc as bacc
import concourse.tile as tile
from concourse import mybir

B, C, L = 32, 512, 4096
N_CORES = 8
NB = B // N_CORES  # batches per core
P = 128            # partitions
NT = 512           # mm2 moving free dim / PSUM bank (f32)
CM = C // P        # 4 channel row-blocks
LN = L // NT       # 8 mm2 output column tiles
LK = L // P        # 32 contraction tiles for mm1



_F32 = mybir.dt.float32
_BF16 = mybir.dt.bfloat16


def build_nc(nb=NB):
    """Build the per-core Bass kernel (nb batches of [C, L] bf16)."""
    nc = bacc.Bacc("TRN2", target_bir_lowering=False, debug=False)
    x_d = nc.dram_tensor("xb", [nb, C, L], _BF16, kind="ExternalInput").ap()
    eta_d = nc.dram_tensor("eta128", [P, 1], _F32, kind="ExternalInput").ap()
    id_d = nc.dram_tensor("ident", [P, P], _BF16, kind="ExternalInput").ap()
    out_d = nc.dram_tensor("out", [nb, C, L], _BF16, kind="ExternalOutput").ap()

    with tile.TileContext(nc) as tc:
        with (
            tc.tile_pool(name="const", bufs=1) as const_pool,
            tc.tile_pool(name="xs", bufs=2 * CM) as x_pool,
            tc.tile_pool(name="xt", bufs=2 * 4) as xT_pool,
            tc.tile_pool(name="ee", bufs=2 * CM) as e_pool,
            tc.tile_pool(name="et", bufs=2 * CM) as et_pool,
            tc.tile_pool(name="sst", bufs=2 * CM) as sstg_pool,
            tc.tile_pool(name="og", bufs=6) as out_pool,
            tc.tile_pool(name="zz", bufs=12 * CM) as stat_pool,
            tc.tile_pool(name="pS", bufs=2, space="PSUM") as pS_pool,
            tc.tile_pool(name="pM", bufs=1, space="PSUM") as pM_pool,
            tc.tile_pool(name="pE", bufs=3, space="PSUM") as pE_pool,
            tc.tile_pool(name="pR", bufs=2, space="PSUM") as pR_pool,
        ):
            ident = const_pool.tile([P, P], _BF16, tag="ident")
            nc.sync.dma_start(ident[:], id_d[:, :])
            eta = const_pool.tile([P, 1], _F32, tag="eta")
            nc.sync.dma_start(eta[:], eta_d[:, :])

            state = {}

            NCH = 4          # transposed-load chunks per batch
            KC = LK // NCH   # k-tiles per chunk

            def emit_loads(b):
                # Chunked whole-rows transposed loads on the SP ring:
                # contiguous source rows give the xbar large descriptors,
                # and chunking lets mm1 start after the first chunk. The
                # resulting (p, j) partitioning of l within a chunk is some
                # fixed bijection; mm1 contracts over ALL of l with both
                # operands from the same tile, so any consistent
                # partitioning is correct.
                xTc = []
                for ci in range(NCH):
                    t = xT_pool.tile([P, KC, C], _BF16, tag="xt",
                                     name=f"xT_{b}_{ci}")
                    nc.sync.dma_start(
                        t[:], x_d[b, :, ci * KC * P:(ci + 1) * KC * P],
                        transpose=True)
                    xTc.append(t)
                # x row-blocks on the ACT ring (independent FIFO from the
                # transposes; needed only from mm2 onwards).
                xs = []
                for m in range(CM):
                    t = x_pool.tile([P, L], _BF16, tag="xs",
                                    name=f"x_{b}_{m}")
                    nc.scalar.dma_start(t[:], x_d[b, m * P:(m + 1) * P, :])
                    xs.append(t)
                state[b] = {"xs": xs, "xTc": xTc}

            def emit_mm1_softmax(b):
                st = state[b]
                xTc = st["xTc"]

                def xt_tile(k):
                    return xTc[k // KC][:, k % KC, :]
                E = [e_pool.tile([P, C], _BF16, tag="ee",
                                 name=f"E_{b}_{m}") for m in range(CM)]
                ET = [et_pool.tile([P, C], _BF16, tag="et",
                                   name=f"ET_{b}_{d}") for d in range(CM)]
                # E^T staging: two 1-bank PSUM tiles, each holding two
                # [P, C] bf16 row-blocks of E^T.
                pEt = [pE_pool.tile([P, 2 * C], _BF16, tag="pE",
                                    name=f"pE_{b}_{h}") for h in range(2)]

                def pE_region(dm, cmi):
                    return pEt[dm // 2][:, (dm % 2) * C + cmi * P:
                                        (dm % 2) * C + (cmi + 1) * P]

                # mirror staging: one 1-bank PSUM tile holding the lower
                # (mirrored) S blocks for row-blocks 1..3, packed:
                # m=1 -> [0:128), m=2 -> [128:384), m=3 -> [384:768)
                pM = pM_pool.tile([P, 6 * P], _BF16, tag="pM",
                                  name=f"pM_{b}")
                pm_off = {1: 0, 2: P, 3: 3 * P}

                def pM_region(m, lo_blk, hi_blk):
                    return pM[:, pm_off[m] + lo_blk * P:
                              pm_off[m] + hi_blk * P]
                svec = []
                for m in range(CM):
                    lo = m * P
                    ps = pS_pool.tile([P, C], _F32, tag="pS")
                    for k in range(LK):
                        nc.tensor.matmul(
                            ps[:, lo:C],
                            xt_tile(k)[:, lo:lo + P],
                            xt_tile(k)[:, lo:C],
                            start=(k == 0),
                            stop=(k == LK - 1),
                        )
                    # rowmin over the upper (computed) region
                    rmin = stat_pool.tile([P, 1], _F32, tag="zz")
                    nc.vector.tensor_reduce(
                        rmin[:], ps[:, lo:C], axis=mybir.AxisListType.X,
                        op=mybir.AluOpType.min)
                    if m > 0:
                        # min over the mirrored (lower) region, then combine
                        rlo = stat_pool.tile([P, 1], _F32, tag="zz")
                        nc.vector.tensor_reduce(
                            rlo[:], pM_region(m, 0, m),
                            axis=mybir.AxisListType.X,
                            op=mybir.AluOpType.min)
                        rc = stat_pool.tile([P, 1], _F32, tag="zz")
                        nc.vector.tensor_tensor(
                            rc[:], rmin[:], rlo[:], op=mybir.AluOpType.min)
                        rmin = rc
                    # stage off-diagonal upper blocks (bf16) and mirror them
                    if m < CM - 1:
                        w = C - (m + 1) * P
                        sb = sstg_pool.tile([P, (CM - 1) * P], _BF16,
                                            tag="sst")
                        if m % 2 == 0:
                            nc.vector.tensor_copy(
                                sb[:, 0:w], ps[:, (m + 1) * P:C])
                        else:
                            nc.scalar.copy(sb[:, 0:w], ps[:, (m + 1) * P:C])
                        for m2 in range(m + 1, CM):
                            nc.tensor.transpose(
                                pM_region(m2, m, m + 1),
                                sb[:, (m2 - m - 1) * P:(m2 - m) * P],
                                ident[:],
                            )
                    # E row-block m: exp(rowmin - S), Z accumulated free
                    zhi = stat_pool.tile([P, 1], _F32, tag="zz")
                    nc.scalar.activation(
                        E[m][:, lo:C], ps[:, lo:C],
                        mybir.ActivationFunctionType.Exp,
                        bias=rmin[:], scale=-1.0, accum_out=zhi[:])
                    if m > 0:
                        zlo = stat_pool.tile([P, 1], _F32, tag="zz")
                        nc.scalar.activation(
                            E[m][:, 0:lo], pM_region(m, 0, m),
                            mybir.ActivationFunctionType.Exp,
                            bias=rmin[:], scale=-1.0, accum_out=zlo[:])
                        z = stat_pool.tile([P, 1], _F32, tag="zz")
                        nc.vector.tensor_tensor(
                            z[:], zhi[:], zlo[:], op=mybir.AluOpType.add)
                    else:
                        z = zhi
                    r = stat_pool.tile([P, 1], _F32, tag="zz")
                    nc.vector.reciprocal(r[:], z[:])
                    s = stat_pool.tile([P, 1], _F32, tag="zz")
                    nc.vector.tensor_tensor(
                        s[:], eta[:], r[:], op=mybir.AluOpType.mult)
                    svec.append(s)
                    # E^T blocks sourced from this row-block
                    for dm in range(CM):
                        nc.tensor.transpose(
                            pE_region(dm, m),
                            E[m][:, dm * P:(dm + 1) * P],
                            ident[:],
                        )
                for dm in range(CM):
                    src = pEt[dm // 2][:, (dm % 2) * C:(dm % 2 + 1) * C]
                    if dm % 2 == 0:
                        nc.vector.tensor_copy(ET[dm][:], src)
                    else:
                        nc.scalar.copy(ET[dm][:], src)
                st["ET"] = ET
                st["svec"] = svec

            def emit_mm2_epilogue(b):
                st = state[b]
                xs, ET, svec = st["xs"], st["ET"], st["svec"]
                cnt = 0
                for m in range(CM):
                    og = out_pool.tile([P, L], _BF16, tag="og",
                                       name=f"og_{b}_{m}")
                    for n in range(LN):
                        pr = pR_pool.tile([P, NT], _F32, tag="pR")
                        for k in range(CM):
                            nc.tensor.matmul(
                                pr[:],
                                ET[k][:, m * P:(m + 1) * P],
                                xs[k][:, n * NT:(n + 1) * NT],
                                start=(k == 0),
                                stop=(k == CM - 1),
                            )
                        sl = og[:, n * NT:(n + 1) * NT]
                        # t = s * R  (PSUM f32 -> SBUF bf16)
                        if cnt % 8 < 5:
                            nc.scalar.mul(sl, pr[:], svec[m][:])
                        else:
                            nc.vector.tensor_scalar_mul(sl, pr[:],
                                                        svec[m][:])
                        # t += x  (SBUF bf16)
                        if cnt % 8 < 3:
                            nc.gpsimd.tensor_tensor(
                                sl, sl, xs[m][:, n * NT:(n + 1) * NT],
                                op=mybir.AluOpType.add)
                        else:
                            nc.vector.tensor_tensor(
                                sl, sl, xs[m][:, n * NT:(n + 1) * NT],
                                op=mybir.AluOpType.add)
                        cnt += 1
                    nc.scalar.dma_start(out_d[b, m * P:(m + 1) * P, :],
                                        og[:])
                del st["xs"], st["xTc"], st["ET"]

            emit_loads(0)
            for b in range(nb):
                emit_mm1_softmax(b)
                if b + 1 < nb:
                    emit_loads(b + 1)
                emit_mm2_epilogue(b)
    nc.compile()
    return nc


_NC_CACHE = {}


def _get_nc():
    if "nc" not in _NC_CACHE:
        _NC_CACHE["nc"] = build_nc()
    return _NC_CACHE["nc"]


def make_in_maps(minibatch: np.ndarray, eta: np.ndarray):
    eta128 = np.ascontiguousarray(
        np.broadcast_to(eta.reshape(1, 1).astype(np.float32), (P, 1)))
    ident = np.eye(P, dtype=ml_dtypes.bfloat16)
    xb = minibatch.astype(ml_dtypes.bfloat16)
    in_maps = []
    for i in range(N_CORES):
        in_maps.append({
            "xb": np.ascontiguousarray(xb[i * NB:(i + 1) * NB]),
            "eta128": eta128,
            "ident": ident,
        })
    return in_maps


def kernel(minibatch: np.ndarray, eta: np.ndarray) -> np.ndarray:
    from concourse.bass_utils import run_bass_kernel_spmd

    assert minibatch.shape == (B, C, L)
    nc = _get_nc()
    in_maps = make_in_maps(minibatch, eta)
    res = run_bass_kernel_spmd(nc, in_maps, core_ids=list(range(N_CORES)))
    out = np.concatenate([res.results[i]["out"] for i in range(N_CORES)],
                         axis=0)
    return out.astype(np.float32)


# revision 26
# speedup vs baseline: 1.2109x; 1.2109x over previous
"""Trainium2 Bass kernel for nn_ConvchannelAttentionBlock.

reference (per batch b):
    S      = x @ x.T                      (C x C, symmetric; contraction over L)
    probs  = softmax(rowmax(S) - S)       (shift-invariant: == softmax(-S))
    read   = probs @ x                    (C x L)
    out    = eta * read + x

Sharding: data-parallel over B. Each of the 8 cores gets 4 batches and
runs an identical NEFF (SPMD) on its shard; outputs are concatenated.

Key structure (v2):
  * x is staged to DRAM in bf16 (working precision); output is written
    bf16 and upcast on the host. Residual error ~0.3% rel, well under
    the 2e-2 gate.
  * xT (the [L, C]-layout copy needed for the S = xT.T @ xT contraction
    over L) is loaded straight from DRAM with the DMA xbar transpose --
    zero PE-transpose / zero cast work on-chip.
  * mm1 computes only upper-triangular blocks of S (symmetric); S stays
    in f32 PSUM.  Lower blocks are mirrored with 6 small PE transposes
    per batch (via a bf16 staging copy of the off-diagonal blocks).
  * softmax: rowmin over the PSUM regions on DVE; E = exp(rowmin - S)
    reads PSUM f32 directly on ACT with fused row-sum accumulators
    (Z comes for free); s = eta/Z on DVE.
  * E^T for mm2's stationary operand via 16 PE transposes per batch.
  * mm2 epilogue: out = s * R + x with s = eta/Z, the per-tile mul/add
    split across ACT/DVE/GPSIMD; result staged bf16, DMA'd in 1MB rows.
"""

import sys

if "/opt/trn_rl_repo" not in sys.path:
    sys.path.insert(0, "/opt/trn_rl_repo")

import numpy as np
import ml_dtypes

import concourse.bacc as bacc
import concourse.tile as tile
from concourse import mybir

B, C, L = 32, 512, 4096
N_CORES = 8
NB = B // N_CORES  # batches per core
P = 128            # partitions
NT = 512           # mm2 moving free dim / PSUM bank (f32)
CM = C // P        # 4 channel row-blocks
LN = L // NT       # 8 mm2 output column tiles
LK = L // P        # 32 contraction tiles for mm1



_F32 = mybir.dt.float32
_BF16 = mybir.dt.bfloat16


def build_nc(nb=NB):
    """Build the per-core Bass kernel (nb batches of [C, L] bf16)."""
    nc = bacc.Bacc("TRN2", target_bir_lowering=False, debug=False)
    x_d = nc.dram_tensor("xb", [nb, C, L], _BF16, kind="ExternalInput").ap()
    eta_d = nc.dram_tensor("eta128", [P, 1], _F32, kind="ExternalInput").ap()
    id_d = nc.dram_tensor("ident", [P, P], _BF16, kind="ExternalInput").ap()
    out_d = nc.dram_tensor("out", [nb, C, L], _BF16, kind="ExternalOutput").ap()

    with tile.TileContext(nc) as tc:
        with (
            tc.tile_pool(name="const", bufs=1) as const_pool,
            tc.tile_pool(name="xs", bufs=2 * CM) as x_pool,
            tc.tile_pool(name="xt", bufs=2 * 4) as xT_pool,
            tc.tile_pool(name="ee", bufs=2 * CM) as e_pool,
            tc.tile_pool(name="et", bufs=2 * CM) as et_pool,
            tc.tile_pool(name="sst", bufs=2 * CM) as sstg_pool,
            tc.tile_pool(name="og", bufs=6) as out_pool,
            tc.tile_pool(name="zz", bufs=12 * CM) as stat_pool,
            tc.tile_pool(name="pS", bufs=2, space="PSUM") as pS_pool,
            tc.tile_pool(name="pM", bufs=1, space="PSUM") as pM_pool,
            tc.tile_pool(name="pE", bufs=3, space="PSUM") as pE_pool,
            tc.tile_pool(name="pR", bufs=2, space="PSUM") as pR_pool,
        ):
            ident = const_pool.tile([P, P], _BF16, tag="ident")
            nc.sync.dma_start(ident[:], id_d[:, :])
            eta = const_pool.tile([P, 1], _F32, tag="eta")
            nc.sync.dma_start(eta[:], eta_d[:, :])

            state = {}

            NCH = 4          # transposed-load chunks per batch
            KC = LK // NCH   # k-tiles per chunk

            def emit_loads(b):
                # Chunked whole-rows transposed loads on the SP ring:
                # contiguous source rows give the xbar large descriptors,
                # and chunking lets mm1 start after the first chunk. The
                # resulting (p, j) partitioning of l within a chunk is some
                # fixed bijection; mm1 contracts over ALL of l with both
                # operands from the same tile, so any consistent
                # partitioning is correct.
                xTc = []
                for ci in range(NCH):
                    t = xT_pool.tile([P, KC, C], _BF16, tag="xt",
                                     name=f"xT_{b}_{ci}")
                    nc.sync.dma_start(
                        t[:], x_d[b, :, ci * KC * P:(ci + 1) * KC * P],
                        transpose=True)
                    xTc.append(t)
                # x row-blocks (needed only from mm2 onwards, so issued
                # after the transposes on the same ring).
                xs = []
                for m in range(CM):
                    t = x_pool.tile([P, L], _BF16, tag="xs",
                                    name=f"x_{b}_{m}")
                    nc.sync.dma_start(t[:], x_d[b, m * P:(m + 1) * P, :])
                    xs.append(t)
                state[b] = {"xs": xs, "xTc": xTc}

            def emit_mm1_softmax(b):
                st = state[b]
                xTc = st["xTc"]

                def xt_tile(k):
                    return xTc[k // KC][:, k % KC, :]
                E = [e_pool.tile([P, C], _BF16, tag="ee",
                                 name=f"E_{b}_{m}") for m in range(CM)]
                ET = [et_pool.tile([P, C], _BF16, tag="et",
                                   name=f"ET_{b}_{d}") for d in range(CM)]
                # E^T staging: two 1-bank PSUM tiles, each holding two
                # [P, C] bf16 row-blocks of E^T.
                pEt = [pE_pool.tile([P, 2 * C], _BF16, tag="pE",
                                    name=f"pE_{b}_{h}") for h in range(2)]

                def pE_region(dm, cmi):
                    return pEt[dm // 2][:, (dm % 2) * C + cmi * P:
                                        (dm % 2) * C + (cmi + 1) * P]

                # mirror staging: one 1-bank PSUM tile holding the lower
                # (mirrored) S blocks for row-blocks 1..3, packed:
                # m=1 -> [0:128), m=2 -> [128:384), m=3 -> [384:768)
                pM = pM_pool.tile([P, 6 * P], _BF16, tag="pM",
                                  name=f"pM_{b}")
                pm_off = {1: 0, 2: P, 3: 3 * P}

                def pM_region(m, lo_blk, hi_blk):
                    return pM[:, pm_off[m] + lo_blk * P:
                              pm_off[m] + hi_blk * P]
                svec = []
                for m in range(CM):
                    lo = m * P
                    ps = pS_pool.tile([P, C], _F32, tag="pS")
                    for k in range(LK):
                        nc.tensor.matmul(
                            ps[:, lo:C],
                            xt_tile(k)[:, lo:lo + P],
                            xt_tile(k)[:, lo:C],
                            start=(k == 0),
                            stop=(k == LK - 1),
                        )
                    # rowmin over the upper (computed) region
                    rmin = stat_pool.tile([P, 1], _F32, tag="zz")
                    nc.vector.tensor_reduce(
                        rmin[:], ps[:, lo:C], axis=mybir.AxisListType.X,
                        op=mybir.AluOpType.min)
                    if m > 0:
                        # min over the mirrored (lower) region, then combine
                        rlo = stat_pool.tile([P, 1], _F32, tag="zz")
                        nc.vector.tensor_reduce(
                            rlo[:], pM_region(m, 0, m),
                            axis=mybir.AxisListType.X,
                            op=mybir.AluOpType.min)
                        rc = stat_pool.tile([P, 1], _F32, tag="zz")
                        nc.vector.tensor_tensor(
                            rc[:], rmin[:], rlo[:], op=mybir.AluOpType.min)
                        rmin = rc
                    # stage off-diagonal upper blocks (bf16) and mirror them
                    if m < CM - 1:
                        w = C - (m + 1) * P
                        sb = sstg_pool.tile([P, (CM - 1) * P], _BF16,
                                            tag="sst")
                        if m % 2 == 0:
                            nc.vector.tensor_copy(
                                sb[:, 0:w], ps[:, (m + 1) * P:C])
                        else:
                            nc.scalar.copy(sb[:, 0:w], ps[:, (m + 1) * P:C])
                        for m2 in range(m + 1, CM):
                            nc.tensor.transpose(
                                pM_region(m2, m, m + 1),
                                sb[:, (m2 - m - 1) * P:(m2 - m) * P],
                                ident[:],
                            )
                    # E row-block m: exp(rowmin - S), Z accumulated free
                    zhi = stat_pool.tile([P, 1], _F32, tag="zz")
                    nc.scalar.activation(
                        E[m][:, lo:C], ps[:, lo:C],
                        mybir.ActivationFunctionType.Exp,
                        bias=rmin[:], scale=-1.0, accum_out=zhi[:])
                    if m > 0:
                        zlo = stat_pool.tile([P, 1], _F32, tag="zz")
                        nc.scalar.activation(
                            E[m][:, 0:lo], pM_region(m, 0, m),
                            mybir.ActivationFunctionType.Exp,
                            bias=rmin[:], scale=-1.0, accum_out=zlo[:])
                        z = stat_pool.tile([P, 1], _F32, tag="zz")
                        nc.vector.tensor_tensor(
                            z[:], zhi[:], zlo[:], op=mybir.AluOpType.add)
                    else:
                        z = zhi
                    r = stat_pool.tile([P, 1], _F32, tag="zz")
                    nc.vector.reciprocal(r[:], z[:])
                    s = stat_pool.tile([P, 1], _F32, tag="zz")
                    nc.vector.tensor_tensor(
                        s[:], eta[:], r[:], op=mybir.AluOpType.mult)
                    svec.append(s)
                    # E^T blocks sourced from this row-block
                    for dm in range(CM):
                        nc.tensor.transpose(
                            pE_region(dm, m),
                            E[m][:, dm * P:(dm + 1) * P],
                            ident[:],
                        )
                for dm in range(CM):
                    src = pEt[dm // 2][:, (dm % 2) * C:(dm % 2 + 1) * C]
                    if dm % 2 == 0:
                        nc.vector.tensor_copy(ET[dm][:], src)
                    else:
                        nc.scalar.copy(ET[dm][:], src)
                st["ET"] = ET
                st["svec"] = svec

            def emit_mm2_epilogue(b):
                st = state[b]
                xs, ET, svec = st["xs"], st["ET"], st["svec"]
                cnt = 0
                for m in range(CM):
                    og = out_pool.tile([P, L], _BF16, tag="og",
                                       name=f"og_{b}_{m}")
                    for n in range(LN):
                        pr = pR_pool.tile([P, NT], _F32, tag="pR")
                        for k in range(CM):
                            nc.tensor.matmul(
                                pr[:],
                                ET[k][:, m * P:(m + 1) * P],
                                xs[k][:, n * NT:(n + 1) * NT],
                                start=(k == 0),
                                stop=(k == CM - 1),
                            )
                        sl = og[:, n * NT:(n + 1) * NT]
                        # t = s * R  (PSUM f32 -> SBUF bf16)
                        if cnt % 8 < 5:
                            nc.scalar.mul(sl, pr[:], svec[m][:])
                        else:
                            nc.vector.tensor_scalar_mul(sl, pr[:],
                                                        svec[m][:])
                        # t += x  (SBUF bf16)
                        if cnt % 8 < 3:
                            nc.gpsimd.tensor_tensor(
                                sl, sl, xs[m][:, n * NT:(n + 1) * NT],
                                op=mybir.AluOpType.add)
                        else:
                            nc.vector.tensor_tensor(
                                sl, sl, xs[m][:, n * NT:(n + 1) * NT],
                                op=mybir.AluOpType.add)
                        cnt += 1
                    nc.sync.dma_start(out_d[b, m * P:(m + 1) * P, :], og[:])
                del st["xs"], st["xTc"], st["ET"]

            emit_loads(0)
            for b in range(nb):
                emit_mm1_softmax(b)
                if b + 1 < nb:
                    emit_loads(b + 1)
                emit_mm2_epilogue(b)
    nc.compile()
    return nc


_NC_CACHE = {}


def _get_nc():
    if "nc" not in _NC_CACHE:
        _NC_CACHE["nc"] = build_nc()
    return _NC_CACHE["nc"]


def make_in_maps(minibatch: np.ndarray, eta: np.ndarray):
    eta128 = np.ascontiguousarray(
        np.broadcast_to(eta.reshape(1, 1).astype(np.float32), (P, 1)))
    ident = np.eye(P, dtype=ml_dtypes.bfloat16)
    xb = minibatch.astype(ml_dtypes.bfloat16)
    in_maps = []
    for i in range(N_CORES):
        in_maps.append({
            "xb": np.ascontiguousarray(xb[i * NB:(i + 1) * NB]),
            "eta128": eta128,
            "ident": ident,
        })
    return in_maps


def kernel(minibatch: np.ndarray, eta: np.ndarray) -> np.ndarray:
    from concourse.bass_utils import run_bass_kernel_spmd

    assert minibatch.shape == (B, C, L)
    nc = _get_nc()
    in_maps = make_in_maps(minibatch, eta)
    res = run_bass_kernel_spmd(nc, in_maps, core_ids=list(range(N_CORES)))
    out = np.concatenate([res.results[i]["out"] for i in range(N_CORES)],
                         axis=0)
    return out.astype(np.float32)


# revision 28
# speedup vs baseline: 1.3372x; 1.1043x over previous
"""Trainium2 Bass kernel for nn_ConvchannelAttentionBlock.

reference (per batch b):
    S      = x @ x.T                      (C x C, symmetric; contraction over L)
    probs  = softmax(rowmax(S) - S)       (shift-invariant: == softmax(-S))
    read   = probs @ x                    (C x L)
    out    = eta * read + x

Sharding: data-parallel over B. Each of the 8 cores gets 4 batches and
runs an identical NEFF (SPMD) on its shard; outputs are concatenated.

Key structure (v2):
  * x is staged to DRAM in bf16 (working precision); output is written
    bf16 and upcast on the host. Residual error ~0.3% rel, well under
    the 2e-2 gate.
  * xT (the [L, C]-layout copy needed for the S = xT.T @ xT contraction
    over L) is loaded straight from DRAM with the DMA xbar transpose --
    zero PE-transpose / zero cast work on-chip.
  * mm1 computes only upper-triangular blocks of S (symmetric); S stays
    in f32 PSUM.  Lower blocks are mirrored with 6 small PE transposes
    per batch (via a bf16 staging copy of the off-diagonal blocks).
  * softmax: rowmin over the PSUM regions on DVE; E = exp(rowmin - S)
    reads PSUM f32 directly on ACT with fused row-sum accumulators
    (Z comes for free); s = eta/Z on DVE.
  * E^T for mm2's stationary operand via 16 PE transposes per batch.
  * mm2 epilogue: out = s * R + x with s = eta/Z, the per-tile mul/add
    split across ACT/DVE/GPSIMD; result staged bf16, DMA'd in 1MB rows.
"""

import sys

if "/opt/trn_rl_repo" not in sys.path:
    sys.path.insert(0, "/opt/trn_rl_repo")

import numpy as np
import ml_dtypes

import concourse.bacc as bacc
import concourse.tile as tile
from concourse import mybir

B, C, L = 32, 512, 4096
N_CORES = 8
NB = B // N_CORES  # batches per core
P = 128            # partitions
NT = 512           # mm2 moving free dim / PSUM bank (f32)
CM = C // P        # 4 channel row-blocks
LN = L // NT       # 8 mm2 output column tiles
LK = L // P        # 32 contraction tiles for mm1



_F32 = mybir.dt.float32
_BF16 = mybir.dt.bfloat16


def build_nc(nb=NB):
    """Build the per-core Bass kernel (nb batches of [C, L] bf16)."""
    nc = bacc.Bacc("TRN2", target_bir_lowering=False, debug=False)
    x_d = nc.dram_tensor("xb", [nb, C, L], _BF16, kind="ExternalInput").ap()
    eta_d = nc.dram_tensor("eta128", [P, 1], _F32, kind="ExternalInput").ap()
    id_d = nc.dram_tensor("ident", [P, P], _BF16, kind="ExternalInput").ap()
    out_d = nc.dram_tensor("out", [nb, C, L], _BF16, kind="ExternalOutput").ap()

    with tile.TileContext(nc) as tc:
        with (
            tc.tile_pool(name="const", bufs=1) as const_pool,
            tc.tile_pool(name="xs", bufs=2 * CM) as x_pool,
            tc.tile_pool(name="xt", bufs=2) as xT_pool,
            tc.tile_pool(name="ee", bufs=2 * CM) as e_pool,
            tc.tile_pool(name="et", bufs=2 * CM) as et_pool,
            tc.tile_pool(name="sst", bufs=2 * CM) as sstg_pool,
            tc.tile_pool(name="og", bufs=6) as out_pool,
            tc.tile_pool(name="zz", bufs=12 * CM) as stat_pool,
            tc.tile_pool(name="pS", bufs=2, space="PSUM") as pS_pool,
            tc.tile_pool(name="pM", bufs=1, space="PSUM") as pM_pool,
            tc.tile_pool(name="pE", bufs=3, space="PSUM") as pE_pool,
            tc.tile_pool(name="pR", bufs=2, space="PSUM") as pR_pool,
        ):
            ident = const_pool.tile([P, P], _BF16, tag="ident")
            nc.sync.dma_start(ident[:], id_d[:, :])
            eta = const_pool.tile([P, 1], _F32, tag="eta")
            nc.sync.dma_start(eta[:], eta_d[:, :])

            state = {}

            NCH = 1          # transposed-load chunks per batch
            KC = LK // NCH   # k-tiles per chunk

            def emit_loads(b):
                # Chunked whole-rows transposed loads on the SP ring:
                # contiguous source rows give the xbar large descriptors,
                # and chunking lets mm1 start after the first chunk. The
                # resulting (p, j) partitioning of l within a chunk is some
                # fixed bijection; mm1 contracts over ALL of l with both
                # operands from the same tile, so any consistent
                # partitioning is correct.
                xTc = []
                for ci in range(NCH):
                    t = xT_pool.tile([P, KC, C], _BF16, tag="xt",
                                     name=f"xT_{b}_{ci}")
                    nc.sync.dma_start(
                        t[:], x_d[b, :, ci * KC * P:(ci + 1) * KC * P],
                        transpose=True)
                    xTc.append(t)
                # x row-blocks (needed only from mm2 onwards, so issued
                # after the transposes on the same ring).
                xs = []
                for m in range(CM):
                    t = x_pool.tile([P, L], _BF16, tag="xs",
                                    name=f"x_{b}_{m}")
                    nc.sync.dma_start(t[:], x_d[b, m * P:(m + 1) * P, :])
                    xs.append(t)
                state[b] = {"xs": xs, "xTc": xTc}

            def emit_mm1_softmax(b):
                st = state[b]
                xTc = st["xTc"]

                def xt_tile(k):
                    return xTc[k // KC][:, k % KC, :]
                E = [e_pool.tile([P, C], _BF16, tag="ee",
                                 name=f"E_{b}_{m}") for m in range(CM)]
                ET = [et_pool.tile([P, C], _BF16, tag="et",
                                   name=f"ET_{b}_{d}") for d in range(CM)]
                # E^T staging: two 1-bank PSUM tiles, each holding two
                # [P, C] bf16 row-blocks of E^T.
                pEt = [pE_pool.tile([P, 2 * C], _BF16, tag="pE",
                                    name=f"pE_{b}_{h}") for h in range(2)]

                def pE_region(dm, cmi):
                    return pEt[dm // 2][:, (dm % 2) * C + cmi * P:
                                        (dm % 2) * C + (cmi + 1) * P]

                # mirror staging: one 1-bank PSUM tile holding the lower
                # (mirrored) S blocks for row-blocks 1..3, packed:
                # m=1 -> [0:128), m=2 -> [128:384), m=3 -> [384:768)
                pM = pM_pool.tile([P, 6 * P], _BF16, tag="pM",
                                  name=f"pM_{b}")
                pm_off = {1: 0, 2: P, 3: 3 * P}

                def pM_region(m, lo_blk, hi_blk):
                    return pM[:, pm_off[m] + lo_blk * P:
                              pm_off[m] + hi_blk * P]
                svec = []
                for m in range(CM):
                    lo = m * P
                    ps = pS_pool.tile([P, C], _F32, tag="pS")
                    for k in range(LK):
                        nc.tensor.matmul(
                            ps[:, lo:C],
                            xt_tile(k)[:, lo:lo + P],
                            xt_tile(k)[:, lo:C],
                            start=(k == 0),
                            stop=(k == LK - 1),
                        )
                    # rowmin over the upper (computed) region
                    rmin = stat_pool.tile([P, 1], _F32, tag="zz")
                    nc.vector.tensor_reduce(
                        rmin[:], ps[:, lo:C], axis=mybir.AxisListType.X,
                        op=mybir.AluOpType.min)
                    if m > 0:
                        # min over the mirrored (lower) region, then combine
                        rlo = stat_pool.tile([P, 1], _F32, tag="zz")
                        nc.vector.tensor_reduce(
                            rlo[:], pM_region(m, 0, m),
                            axis=mybir.AxisListType.X,
                            op=mybir.AluOpType.min)
                        rc = stat_pool.tile([P, 1], _F32, tag="zz")
                        nc.vector.tensor_tensor(
                            rc[:], rmin[:], rlo[:], op=mybir.AluOpType.min)
                        rmin = rc
                    # stage off-diagonal upper blocks (bf16) and mirror them
                    if m < CM - 1:
                        w = C - (m + 1) * P
                        sb = sstg_pool.tile([P, (CM - 1) * P], _BF16,
                                            tag="sst")
                        if m % 2 == 0:
                            nc.vector.tensor_copy(
                                sb[:, 0:w], ps[:, (m + 1) * P:C])
                        else:
                            nc.scalar.copy(sb[:, 0:w], ps[:, (m + 1) * P:C])
                        for m2 in range(m + 1, CM):
                            nc.tensor.transpose(
                                pM_region(m2, m, m + 1),
                                sb[:, (m2 - m - 1) * P:(m2 - m) * P],
                                ident[:],
                            )
                    # E row-block m: exp(rowmin - S), Z accumulated free
                    zhi = stat_pool.tile([P, 1], _F32, tag="zz")
                    nc.scalar.activation(
                        E[m][:, lo:C], ps[:, lo:C],
                        mybir.ActivationFunctionType.Exp,
                        bias=rmin[:], scale=-1.0, accum_out=zhi[:])
                    if m > 0:
                        zlo = stat_pool.tile([P, 1], _F32, tag="zz")
                        nc.scalar.activation(
                            E[m][:, 0:lo], pM_region(m, 0, m),
                            mybir.ActivationFunctionType.Exp,
                            bias=rmin[:], scale=-1.0, accum_out=zlo[:])
                        z = stat_pool.tile([P, 1], _F32, tag="zz")
                        nc.vector.tensor_tensor(
                            z[:], zhi[:], zlo[:], op=mybir.AluOpType.add)
                    else:
                        z = zhi
                    r = stat_pool.tile([P, 1], _F32, tag="zz")
                    nc.vector.reciprocal(r[:], z[:])
                    s = stat_pool.tile([P, 1], _F32, tag="zz")
                    nc.vector.tensor_tensor(
                        s[:], eta[:], r[:], op=mybir.AluOpType.mult)
                    svec.append(s)
                    # E^T blocks sourced from this row-block
                    for dm in range(CM):
                        nc.tensor.transpose(
                            pE_region(dm, m),
                            E[m][:, dm * P:(dm + 1) * P],
                            ident[:],
                        )
                for dm in range(CM):
                    src = pEt[dm // 2][:, (dm % 2) * C:(dm % 2 + 1) * C]
                    if dm % 2 == 0:
                        nc.vector.tensor_copy(ET[dm][:], src)
                    else:
                        nc.scalar.copy(ET[dm][:], src)
                st["ET"] = ET
                st["svec"] = svec

            def emit_mm2_epilogue(b):
                st = state[b]
                xs, ET, svec = st["xs"], st["ET"], st["svec"]
                cnt = 0
                for m in range(CM):
                    og = out_pool.tile([P, L], _BF16, tag="og",
                                       name=f"og_{b}_{m}")
                    for n in range(LN):
                        pr = pR_pool.tile([P, NT], _F32, tag="pR")
                        for k in range(CM):
                            nc.tensor.matmul(
                                pr[:],
                                ET[k][:, m * P:(m + 1) * P],
                                xs[k][:, n * NT:(n + 1) * NT],
                                start=(k == 0),
                                stop=(k == CM - 1),
                            )
                        sl = og[:, n * NT:(n + 1) * NT]
                        # t = s * R  (PSUM f32 -> SBUF bf16)
                        if cnt % 8 < 5:
                            nc.scalar.mul(sl, pr[:], svec[m][:])
                        else:
                            nc.vector.tensor_scalar_mul(sl, pr[:],
                                                        svec[m][:])
                        # t += x  (SBUF bf16)
                        if cnt % 8 < 3:
                            nc.gpsimd.tensor_tensor(
                                sl, sl, xs[m][:, n * NT:(n + 1) * NT],
                                op=mybir.AluOpType.add)
                        else:
                            nc.vector.tensor_tensor(
                                sl, sl, xs[m][:, n * NT:(n + 1) * NT],
                                op=mybir.AluOpType.add)
                        cnt += 1
                    nc.sync.dma_start(out_d[b, m * P:(m + 1) * P, :], og[:])
                del st["xs"], st["xTc"], st["ET"]

            emit_loads(0)
            for b in range(nb):
                emit_mm1_softmax(b)
                if b + 1 < nb:
                    emit_loads(b + 1)
                emit_mm2_epilogue(b)
    nc.compile()
    return nc


_NC_CACHE = {}


def _get_nc():
    if "nc" not in _NC_CACHE:
        _NC_CACHE["nc"] = build_nc()
    return _NC_CACHE["nc"]


def make_in_maps(minibatch: np.ndarray, eta: np.ndarray):
    eta128 = np.ascontiguousarray(
        np.broadcast_to(eta.reshape(1, 1).astype(np.float32), (P, 1)))
    ident = np.eye(P, dtype=ml_dtypes.bfloat16)
    xb = minibatch.astype(ml_dtypes.bfloat16)
    in_maps = []
    for i in range(N_CORES):
        in_maps.append({
            "xb": np.ascontiguousarray(xb[i * NB:(i + 1) * NB]),
            "eta128": eta128,
            "ident": ident,
        })
    return in_maps


def kernel(minibatch: np.ndarray, eta: np.ndarray) -> np.ndarray:
    from concourse.bass_utils import run_bass_kernel_spmd

    assert minibatch.shape == (B, C, L)
    nc = _get_nc()
    in_maps = make_in_maps(minibatch, eta)
    res = run_bass_kernel_spmd(nc, in_maps, core_ids=list(range(N_CORES)))
    out = np.concatenate([res.results[i]["out"] for i in range(N_CORES)],
                         axis=0)
    return out.astype(np.float32)
